# revision 4
# baseline (speedup 1.0000x reference)
"""3D Haar DWT low-pass (DWT3DTiny) Trainium2 kernel.

The reference applies the Haar rec_lo filter [s, s] (s = sqrt(2)/2) with
stride-2 downsampling along t, h, w for every channel.  That is exactly a
2x2x2 box sum scaled by s^3 = 2**-1.5:

    out[ts, hs, ws, c] = 2**-1.5 * sum_{dt,dh,dw in {0,1}} x[2ts+dt, 2hs+dh, 2ws+dw, c]

Sharding: along t (pure data-parallel, t-pairs never cross a core
boundary since 32 / 8 = 4 rows per core), contiguous host-side slices.

v4 design (from NTFF packet analysis; the 16 SDMA engines are ~97%
occupied in-span, per-packet throughput grows with descriptor size:
2 KiB -> 24.4, 8 KiB -> 26.0, 32 KiB -> 26.6 GB/s/engine):
  * chunk = (t-pair, 256-h-row block); partition p holds h rows
    (2p, 2p+1), full width -> 32 KiB contiguous descriptors for the
    first t row (a), 8 KiB for the halved second row (b halves);
  * b is loaded in two half-width tiles so the serial DVE chain that
    becomes runnable only when a b-tile lands is halved;
  * the scale 2**-1.5 is folded into the adds: after ha = a0 + a1 the
    tile is scaled in place (tensor_scalar runs 2x for fp32 in SBUF),
    and the t-add is scalar_tensor_tensor (hb * s) + ha' -> every
    output term carries s exactly once; no ACT muls remain;
  * w-pair adds run on the otherwise idle Pool engine (GpSimd), off
    the DVE critical path; ACT only issues the stores (8 KiB descs);
  * the final chunk's a-row is loaded + reduced at the very start of
    the run (DVE is idle there anyway), and its b is loaded as one
    half + graduated pieces [96,64,48,32,16] wi so the post-last-load
    drain is just the 16-wi chain;
  * loads on the SP HWDGE ring, stores on the ACT ring;
  * dead const-tile memsets stripped from the init preamble (~9 us of
    GpSimd startup the all-engine barrier otherwise waits on).
"""

import numpy as np

import concourse.bacc as bacc
import concourse.mybir as mybir
from concourse.bass_utils import run_bass_kernel_spmd
from concourse.tile import TileContext

N_CORES = 8
T, H, W, C = 32, 512, 512, 8
TS = T // N_CORES  # t rows per core
SCALE = float(2.0 ** -1.5)
TAIL_WI = [96, 64, 48, 32, 16]  # graduated pieces of the final b half
WC = W * C  # 4096 f32 per h row
HWC = WC // 2  # half-row, 2048 f32

_CACHE: dict = {}


def _build_nc() -> bacc.Bacc:
    nc = bacc.Bacc("TRN2", target_bir_lowering=False)
    x = nc.dram_tensor("x", [TS, H, W, C], mybir.dt.float32, kind="ExternalInput")
    y = nc.dram_tensor(
        "y", [TS // 2, H // 2, W // 2, C], mybir.dt.float32, kind="ExternalOutput"
    )

    # h = gb*256 + p*2 + two; rows 2p, 2p+1 full-width are adjacent in HBM.
    xq = x.rearrange("t (gb p two) w c -> t gb p two (w c)", p=128, two=2)
    # output row g = gb*128 + p: 256 v * 8 c = 8 KiB contiguous per partition
    yq = y.rearrange("s (gb p) w c -> s gb p (w c)", p=128)

    add = mybir.AluOpType.add
    mult = mybir.AluOpType.mult

    chunks = [(tp, gb) for tp in range(TS // 2) for gb in range(H // 256)]
    LAST = len(chunks) - 1

    with TileContext(nc) as tc:
        with (
            tc.tile_pool(name="pin", bufs=2) as pin,
            tc.tile_pool(name="pw", bufs=2) as pw,
            tc.tile_pool(name="ptl", bufs=1) as pt,
        ):
            def wadd(engine, src, n, wtile, ydst):
                # w-pair add (wi = v*2 + dw) of pre-scaled data, then store
                hv = src.rearrange("p (v two c) -> p v two c", two=2, c=C)
                wv = wtile.rearrange("p (v c) -> p v c", c=C)
                engine.tensor_add(out=wv[:], in0=hv[:, :, 0], in1=hv[:, :, 1])
                nc.scalar.dma_start(out=ydst, in_=wtile[:])

            def half_chain(bh, a, j, w_engine, wtile, wslice, ydst):
                # hb = b0 + b1 ; t' = hb*s + ha'  (everything carries s once)
                nc.vector.tensor_add(out=bh[:, 0], in0=bh[:, 0], in1=bh[:, 1])
                nc.vector.scalar_tensor_tensor(
                    out=bh[:, 0], in0=bh[:, 0], scalar=SCALE,
                    in1=a[:, 0, j * HWC : (j + 1) * HWC], op0=mult, op1=add,
                )
                wadd(w_engine, bh[:, 0], HWC, wslice, ydst)

            # --- tail a row: load + reduce + pre-scale at run start -------
            ttp, tgb = chunks[LAST]
            ta0 = pt.tile([128, WC], mybir.dt.float32, tag="ta0")
            ta1 = pin.tile([128, 2, HWC], mybir.dt.float32, tag="bh0")
            nc.sync.dma_start(out=ta0[:], in_=xq[2 * ttp, tgb, :, 0])
            nc.sync.dma_start(
                out=ta1[:], in_=xq[2 * ttp, tgb, :, 1].rearrange("p (a b) -> p a b", a=2)
            )
            nc.vector.tensor_add(
                out=ta0[:], in0=ta0[:], in1=ta1.rearrange("p a b -> p (a b)")[:]
            )
            nc.vector.tensor_scalar_mul(ta0[:], ta0[:], SCALE)

            # --- bulk chunks ---------------------------------------------
            for ci, (tp, gb) in enumerate(chunks[:-1]):
                a = pin.tile([128, 2, WC], mybir.dt.float32, tag="a")
                nc.sync.dma_start(out=a[:], in_=xq[2 * tp, gb])
                nc.vector.tensor_add(out=a[:, 0], in0=a[:, 0], in1=a[:, 1])
                nc.vector.tensor_scalar_mul(a[:, 0], a[:, 0], SCALE)
                ws = pw.tile([128, WC // 2], mybir.dt.float32, tag="w")
                for j in range(2):
                    bh = pin.tile([128, 2, HWC], mybir.dt.float32, tag=f"bh{j}")
                    nc.sync.dma_start(
                        out=bh[:],
                        in_=xq[2 * tp + 1, gb, :, :, j * HWC : (j + 1) * HWC],
                    )
                    half_chain(
                        bh, a, j, nc.gpsimd, ws,
                        ws[:, j * (HWC // 2) : (j + 1) * (HWC // 2)],
                        yq[tp, gb, :, j * (HWC // 2) : (j + 1) * (HWC // 2)],
                    )

            # --- tail chunk: first half normal, then graduated pieces ----
            tb0 = pin.tile([128, 2, HWC], mybir.dt.float32, tag="bh0")
            nc.sync.dma_start(out=tb0[:], in_=xq[2 * ttp + 1, tgb, :, :, 0:HWC])
            nc.vector.tensor_add(out=tb0[:, 0], in0=tb0[:, 0], in1=tb0[:, 1])
            nc.vector.scalar_tensor_tensor(
                out=tb0[:, 0], in0=tb0[:, 0], scalar=SCALE,
                in1=ta0[:, 0:HWC], op0=mult, op1=add,
            )
            tw0 = pt.tile([128, HWC // 2], mybir.dt.float32, tag="tw0")
            wadd(nc.gpsimd, tb0[:, 0], HWC, tw0[:], yq[ttp, tgb, :, 0 : HWC // 2])

            w0 = W // 2
            for k, wi in enumerate(TAIL_WI):
                wc = wi * C
                bp = pt.tile([128, 2, wc], mybir.dt.float32, tag=f"tb{k}")
                nc.sync.dma_start(
                    out=bp[:],
                    in_=xq[2 * ttp + 1, tgb, :, :, w0 * C : (w0 + wi) * C],
                )
                nc.vector.tensor_add(out=bp[:, 0], in0=bp[:, 0], in1=bp[:, 1])
                nc.vector.scalar_tensor_tensor(
                    out=bp[:, 0], in0=bp[:, 0], scalar=SCALE,
                    in1=ta0[:, w0 * C : (w0 + wi) * C], op0=mult, op1=add,
                )
                wt = pt.tile([128, wc // 2], mybir.dt.float32, tag=f"tw{k + 1}")
                w_eng = nc.vector if k == len(TAIL_WI) - 1 else nc.gpsimd
                wadd(
                    w_eng, bp[:, 0], wc, wt[:],
                    yq[ttp, tgb, :, (w0 // 2) * C : ((w0 + wi) // 2) * C],
                )
                w0 += wi

    _strip_init_preamble(nc)
    if not nc.is_finalized():
        nc.finalize()  # Bacc.compile: event-sem split (1 wait/inst), reg alloc
    return nc


def _strip_init_preamble(nc) -> None:
    """Drop the four Bass.__init__ const-tile memsets from block 0.  Nothing
    in this kernel reads the const tiles, yet the initial all-engine barrier
    waits on the GpSimd engine executing them, which costs ~9 us of Q7
    startup on HW.  The drains and the all-engine barrier are kept."""
    b0 = nc.main_func.blocks[0]
    b0.instructions[:] = [
        ins for ins in b0.instructions if type(ins).__name__ != "InstMemset"
    ]


def kernel(x) -> np.ndarray:
    x = np.asarray(x, dtype=np.float32)
    assert x.shape == (T, H, W, C), x.shape

    if "nc" not in _CACHE:
        _CACHE["nc"] = _build_nc()
    nc = _CACHE["nc"]

    in_maps = [
        {"x": np.ascontiguousarray(x[i * TS : (i + 1) * TS])} for i in range(N_CORES)
    ]
    res = run_bass_kernel_spmd(nc, in_maps, core_ids=list(range(N_CORES)))
    return np.concatenate([r["y"] for r in res.results], axis=0)


# revision 9
# speedup vs baseline: 1.0398x; 1.0398x over previous
"""3D Haar DWT low-pass (DWT3DTiny) Trainium2 kernel.

The reference applies the Haar rec_lo filter [s, s] (s = sqrt(2)/2) with
stride-2 downsampling along t, h, w for every channel.  That is exactly a
2x2x2 box sum scaled by s^3 = 2**-1.5:

    out[ts, hs, ws, c] = 2**-1.5 * sum_{dt,dh,dw in {0,1}} x[2ts+dt, 2hs+dh, 2ws+dw, c]

Sharding: along t (pure data-parallel, t-pairs never cross a core
boundary since 32 / 8 = 4 rows per core), contiguous host-side slices.

v5 design.  NTFF packet analysis shows the 16 SDMA engines are ~97%
occupied in-span and per-packet throughput grows with descriptor size
(2 KiB -> 24.4, 8 KiB -> 26.0, 32 KiB -> 26.6 GB/s/engine), while the
end of the run is bounded by the serial DVE chain that becomes
runnable only after the last tiles land.  Hence:
  * chunk = (t-pair, 256-h-row block); partition p holds h rows
    (2p, 2p+1) full width -> one 32 KiB contiguous descriptor per
    partition for the first t row (a); the second row (b) is loaded as
    two half-width tiles (8 KiB descriptors) so the post-landing DVE
    chain (hb, t, w) is halved per landing;
  * per half: hb = b0+b1, t = hb+ha, w-pair add; ACT scales the whole
    chunk once and issues one 8 KiB-descriptor store;
  * the final chunk's a-row is loaded first and reduced + pre-scaled
    (tensor_scalar runs 2x for fp32) while DVE is otherwise idle; its
    b comes as one half + graduated pieces [96,64,48,32,16] wi whose
    t-add is scalar_tensor_tensor (hb*s + ha') so the pieces skip the
    ACT mul entirely and the post-last-load drain is just the 16-wi
    chain plus one small store;
  * loads on the SP HWDGE ring, stores on the ACT ring (sharing one
    ring head-of-line blocks loads behind stores);
  * dead const-tile memsets stripped from the init preamble (~9 us of
    GpSimd startup the all-engine barrier otherwise waits on).
Rejected experimentally: w-adds on the GpSimd/Pool engine (2.3x slower
per element and its sem latency lands on the store path: 97.2 us),
uniform small tail pieces (92.2), all-big tail pieces (92.0 with 7.7 us
drain), SWDGE loads, 4 MiB loads with bufs=2 (v1 notes).
"""

import numpy as np

import concourse.bacc as bacc
import concourse.mybir as mybir
from concourse.bass_utils import run_bass_kernel_spmd
from concourse.tile import TileContext

N_CORES = 8
T, H, W, C = 32, 512, 512, 8
TS = T // N_CORES  # t rows per core
SCALE = float(2.0 ** -1.5)
TAIL_WI = [96, 64, 48, 32, 16]  # graduated pieces of the final b half
WC = W * C  # 4096 f32 per h row
HWC = WC // 2  # half row, 2048 f32

_CACHE: dict = {}


def _build_nc() -> bacc.Bacc:
    nc = bacc.Bacc("TRN2", target_bir_lowering=False)
    x = nc.dram_tensor("x", [TS, H, W, C], mybir.dt.float32, kind="ExternalInput")
    y = nc.dram_tensor(
        "y", [TS // 2, H // 2, W // 2, C], mybir.dt.float32, kind="ExternalOutput"
    )

    # h = gb*256 + p*2 + two; rows 2p, 2p+1 full-width are adjacent in HBM.
    xq = x.rearrange("t (gb p two) w c -> t gb p two (w c)", p=128, two=2)
    # output row g = gb*128 + p: 256 v * 8 c = 8 KiB contiguous per partition
    yq = y.rearrange("s (gb p) w c -> s gb p (w c)", p=128)

    add = mybir.AluOpType.add
    mult = mybir.AluOpType.mult

    chunks = [(tp, gb) for tp in range(TS // 2) for gb in range(H // 256)]
    LAST = len(chunks) - 1
    ttp, tgb = chunks[LAST]

    def wadd(src, wdst):
        # w-pair add (wi = v*2 + dw): src [128, n] -> wdst [128, n//2]
        hv = src.rearrange("p (v two c) -> p v two c", two=2, c=C)
        wv = wdst.rearrange("p (v c) -> p v c", c=C)
        nc.vector.tensor_add(out=wv[:], in0=hv[:, :, 0], in1=hv[:, :, 1])

    with TileContext(nc) as tc:
        with (
            tc.tile_pool(name="pin", bufs=2) as pin,
            tc.tile_pool(name="pw", bufs=2) as pw,
            tc.tile_pool(name="ptl", bufs=1) as pt,
        ):
            # --- tail a row first: load, reduce, pre-scale (DVE is idle) --
            ta = pt.tile([128, 2, WC], mybir.dt.float32, tag="ta")
            nc.sync.dma_start(out=ta[:], in_=xq[2 * ttp, tgb])
            nc.vector.tensor_add(out=ta[:, 0], in0=ta[:, 0], in1=ta[:, 1])
            nc.vector.tensor_scalar_mul(ta[:, 0], ta[:, 0], SCALE)

            # --- bulk chunks ---------------------------------------------
            for tp, gb in chunks[:-1]:
                a = pin.tile([128, 2, WC], mybir.dt.float32, tag="a")
                nc.sync.dma_start(out=a[:], in_=xq[2 * tp, gb])
                nc.vector.tensor_add(out=a[:, 0], in0=a[:, 0], in1=a[:, 1])
                ws = pw.tile([128, WC // 2], mybir.dt.float32, tag="w")
                for j in range(2):
                    bh = pin.tile([128, 2, HWC], mybir.dt.float32, tag=f"bh{j}")
                    nc.sync.dma_start(
                        out=bh[:],
                        in_=xq[2 * tp + 1, gb, :, :, j * HWC : (j + 1) * HWC],
                    )
                    nc.vector.tensor_add(out=bh[:, 0], in0=bh[:, 0], in1=bh[:, 1])
                    nc.vector.tensor_add(
                        out=bh[:, 0], in0=bh[:, 0],
                        in1=a[:, 0, j * HWC : (j + 1) * HWC],
                    )
                    wadd(bh[:, 0], ws[:, j * (HWC // 2) : (j + 1) * (HWC // 2)])
                nc.scalar.mul(ws[:], ws[:], SCALE)
                nc.scalar.dma_start(out=yq[tp, gb], in_=ws[:])

            # --- tail chunk: one normal half, then graduated pieces ------
            def tail_piece(w0, wi, tagk, bpool=None, btag=None):
                wc = wi * C
                bp = (bpool or pt).tile(
                    [128, 2, wc], mybir.dt.float32, tag=btag or f"tb{tagk}"
                )
                nc.sync.dma_start(
                    out=bp[:],
                    in_=xq[2 * ttp + 1, tgb, :, :, w0 * C : (w0 + wi) * C],
                )
                nc.vector.tensor_add(out=bp[:, 0], in0=bp[:, 0], in1=bp[:, 1])
                # t-add with the scale folded in: s*hb + (s*ha) -> final
                nc.vector.scalar_tensor_tensor(
                    out=bp[:, 0], in0=bp[:, 0], scalar=SCALE,
                    in1=ta[:, 0, w0 * C : (w0 + wi) * C], op0=mult, op1=add,
                )
                wt = pt.tile([128, wc // 2], mybir.dt.float32, tag=f"tw{tagk}")
                wadd(bp[:, 0], wt[:])
                nc.scalar.dma_start(
                    out=yq[ttp, tgb, :, (w0 // 2) * C : ((w0 + wi) // 2) * C],
                    in_=wt[:],
                )

            # first half as one piece; its b tile reuses the pin bh0 ring
            # (same shape), so only the graduated pieces need dedicated SBUF
            tail_piece(0, 256, "h0", bpool=pin, btag="bh0")
            w0 = W // 2
            for k, wi in enumerate(TAIL_WI):
                tail_piece(w0, wi, str(k))
                w0 += wi

    _strip_init_preamble(nc)
    if not nc.is_finalized():
        nc.finalize()  # Bacc.compile: event-sem split (1 wait/inst), reg alloc
    return nc


def _strip_init_preamble(nc) -> None:
    """Drop the four Bass.__init__ const-tile memsets from block 0.  Nothing
    in this kernel reads the const tiles, yet the initial all-engine barrier
    waits on the GpSimd engine executing them, which costs ~9 us of Q7
    startup on HW.  The drains and the all-engine barrier are kept."""
    b0 = nc.main_func.blocks[0]
    b0.instructions[:] = [
        ins for ins in b0.instructions if type(ins).__name__ != "InstMemset"
    ]


def kernel(x) -> np.ndarray:
    x = np.asarray(x, dtype=np.float32)
    assert x.shape == (T, H, W, C), x.shape

    if "nc" not in _CACHE:
        _CACHE["nc"] = _build_nc()
    nc = _CACHE["nc"]

    in_maps = [
        {"x": np.ascontiguousarray(x[i * TS : (i + 1) * TS])} for i in range(N_CORES)
    ]
    res = run_bass_kernel_spmd(nc, in_maps, core_ids=list(range(N_CORES)))
    return np.concatenate([r["y"] for r in res.results], axis=0)
